# revision 15
# baseline (speedup 1.0000x reference)
"""Block-diagonal grouped GEMM (GroupLinear) on 8 TRN2 NeuronCores.

Problem: x [8, 2048, 4096] f32, W [4096, 4096] f32 where only the 64
diagonal 64x64 blocks of W are used:
    y[b,s, g*64+o] = sum_i x[b,s, g*64+i] * W[g*64+o, g*64+i]

Strategy:
  - Data-parallel over batch: core b handles x[b] (2048 tokens).
  - Whole device pipeline in fp16 (harness gate is rel_err < 2e-2; fp16
    end-to-end costs ~3e-4): halves HBM traffic vs fp32, PE runs at
    1 cycle/row instead of 4.
  - Host feeds each core xT = x[b].T [4096, 2048] (channel-major) so the
    PE contraction dim is already on partitions; host also packs pairs
    of 64-ch groups into 32 block-diagonal [128i, 128o] weight strips.
  - Per strip: load xT strip [128, 2048] (512KB contiguous, Sync HWDGE
    ring - the Sync sequencer does nothing else so load issue is never
    coupled to compute), 4 matmuls [K=128] x [128, 512] -> PSUM fp32,
    PSUM->SBUF fp16 copies alternating Vector/Scalar engines, one 512KB
    store per strip on the Pool SWDGE ring (its sequencer is otherwise
    idle, so store issue never contends with the copy engines).
  - Weights ride the Scalar HWDGE + Pool SWDGE rings as two separate
    tiles so strip 0 only waits on the first half and the Sync ring
    streams x from t=0.
  - Drain: the last three strips' stores are placed on the by-then-idle
    HWDGE rings (Scalar/Sync) so they don't queue behind the Pool ring's
    store backlog, and the final strip is stored chunk-by-chunk.
  - Host transposes yT back and upcasts.
"""

import numpy as np

import concourse.bacc as bacc
import concourse.mybir as mybir
from concourse.tile import TileContext
from concourse.bass_utils import run_bass_kernel_spmd

B, S, C = 8, 2048, 4096
G, GS = 64, 64            # groups, group size (=in_scale=out_scale)
NSTRIP = C // 128         # 32 strips of 128 channels (2 groups each)
TOK = 512                 # moving-operand free dim (one PSUM bank)
FP16 = mybir.dt.float16
FP32 = mybir.dt.float32


def _build_program():
    nc = bacc.Bacc()
    xt = nc.declare_dram_parameter("xt", [C, S], FP16, isOutput=False)
    wb = nc.declare_dram_parameter("wb", [128, NSTRIP * 128], FP16, isOutput=False)
    yt = nc.declare_dram_parameter("yt", [C, S], FP16, isOutput=True)

    with TileContext(nc) as tc:
        with (
            tc.tile_pool(name="wpool", bufs=2) as wpool,
            tc.tile_pool(name="xpool", bufs=16) as xpool,
            tc.tile_pool(name="opool", bufs=8) as opool,
            tc.tile_pool(name="ppool", bufs=8, space="PSUM") as ppool,
        ):
            half = NSTRIP * 128 // 2
            w_a = wpool.tile([128, half], FP16)
            w_b = wpool.tile([128, half], FP16)
            nc.scalar.dma_start(out=w_a[:], in_=wb[:, :half])
            nc.gpsimd.dma_start(out=w_b[:], in_=wb[:, half:])
            for c in range(NSTRIP):
                x_t = xpool.tile([128, S], FP16)
                nc.sync.dma_start(out=x_t[:], in_=xt[c * 128:(c + 1) * 128, :])
                last = c == NSTRIP - 1
                o_t = opool.tile([128, S], FP16)
                w_h = w_a if c < 16 else w_b
                wcol = (c % 16) * 128
                for tb in range(4):
                    ps = ppool.tile([128, TOK], FP32)
                    nc.tensor.matmul(
                        out=ps[:],
                        lhsT=w_h[:, wcol:wcol + 128],
                        rhs=x_t[:, tb * TOK:(tb + 1) * TOK],
                        start=True,
                        stop=True,
                    )
                    dst = o_t[:, tb * TOK:(tb + 1) * TOK]
                    if (c * 4 + tb) % 2 == 0:
                        nc.vector.tensor_copy(out=dst, in_=ps[:])
                    else:
                        nc.scalar.copy(out=dst, in_=ps[:])
                    if last:
                        # Final strip: store each chunk as soon as its copy
                        # lands, alternating the two idle HWDGE rings.
                        eng = (nc.scalar, nc.sync, nc.scalar, nc.sync)[tb]
                        eng.dma_start(
                            out=yt[c * 128:(c + 1) * 128,
                                   tb * TOK:(tb + 1) * TOK],
                            in_=o_t[:, tb * TOK:(tb + 1) * TOK],
                        )
                if not last:
                    if c >= NSTRIP - 3:
                        eng = nc.scalar     # jump the Pool store backlog
                    else:
                        eng = nc.gpsimd
                    eng.dma_start(
                        out=yt[c * 128:(c + 1) * 128, :], in_=o_t[:]
                    )
    nc.finalize()
    return nc


def _prep_in_maps(x, W):
    # Diagonal blocks: Wdiag[g][o, i] = W[g*64+o, g*64+i]
    Wr = W.reshape(G, GS, G, GS)
    g = np.arange(G)
    WdT = Wr[g, :, g, :].transpose(0, 2, 1).astype(np.float16)   # [g, i, o]
    wb = np.zeros((128, NSTRIP, 128), dtype=np.float16)
    for c in range(NSTRIP):
        wb[0:64, c, 0:64] = WdT[2 * c]
        wb[64:128, c, 64:128] = WdT[2 * c + 1]
    wb = np.ascontiguousarray(wb.reshape(128, NSTRIP * 128))
    xh = x.astype(np.float16)
    return [
        {"xt": np.ascontiguousarray(xh[b].T), "wb": wb}
        for b in range(B)
    ]


def run(x, W, trace=False, **kw):
    x = np.asarray(x, dtype=np.float32)
    W = np.asarray(W, dtype=np.float32)
    nc = _build_program()
    in_maps = _prep_in_maps(x, W)
    res = run_bass_kernel_spmd(nc, in_maps, list(range(B)), trace=trace, **kw)
    y = np.empty((B, S, C), dtype=np.float32)
    for b in range(B):
        y[b] = res.results[b]["yt"].T.astype(np.float32)
    return y, res


def kernel(x, W):
    y, _ = run(x, W, trace=False)
    return y
